# revision 1
# baseline (speedup 1.0000x reference)
"""AttnBlock (GroupNorm + single-head 4096-token attention + proj + residual)
on 8 Trainium2 NeuronCores.

Sharding: core = (batch b = core//4, query-chunk qc = core%4).
Each core redundantly computes GroupNorm stats + full K/V for its batch
(K/V are needed by every query), and attention/proj for its 1024 queries.
No collectives; host slices inputs and concatenates outputs.

All tensors are kept channel-major ("transposed", [C, n]) on chip so that
every matmul is expressible without any on-chip transposes:
  K^T[o,n]  = wk[c,o].T @ h^T[c,n]        (lhsT = wk chunk, rhs = h^T)
  Q^T[o,i]  = wq[c,o].T @ hq^T[c,i]
  V[n,c]    = h^T[c,n-blk].T @ wv[c,:]    (lhsT = h^T chunk, rhs = wv)
  S^T[j,i]  = K^T[o,j-blk].T @ Q^T[o,i]   (accum over 4 o-tiles)
  E = exp(S^T/sqrt(C))  (no max-subtraction; scores are O(1) for this model)
  D[1,i]    = ones[j,1].T @ E             (softmax denominator via PE)
  O^T[c,i]  = V[j,c-blk].T @ E            (accum over all 32 j-tiles in PSUM)
  out^T[o,i]= wproj[c,o].T @ (O^T * (1/D)) + bproj_eff + xq^T
bv is folded into bproj_eff = bproj + bv @ wproj on the host.
Matmuls run as float32r (full PE rate at moving-dim 512, ~fp32 precision).
"""

import os
import sys

import numpy as np

sys.path.insert(0, "/opt/trn_rl_repo")

import concourse.bass as bass
import concourse.bacc as bacc
import concourse.tile as tile
from concourse import mybir
from concourse.bass_utils import run_bass_kernel_spmd

F32 = mybir.dt.float32
F32R = mybir.dt.float32r
AF = mybir.ActivationFunctionType
OP = mybir.AluOpType

B = 2
C = 512
N = 4096          # H*W tokens per batch
NQ = 1024         # queries per core
P = 128
NT = C // P       # 4 channel tiles
NCH = N // 512    # 8 column chunks of x
EPS = 1e-6
SM_SCALE = float(C) ** -0.5
NCORES = 8

_CACHE = {}
USE_CC = True


def _emit(tc, t):
    """Emit the whole per-core kernel. `t` maps name -> DRAM tensor handle."""
    nc = tc.nc
    r = lambda ap: ap.bitcast(F32R)
    NJ = N // P  # 32 j-tiles

    with (
        tc.tile_pool(name="consts", bufs=1) as consts,
        tc.tile_pool(name="ktpool", bufs=1) as ktpool,
        tc.tile_pool(name="vpool", bufs=1) as vpool,
        tc.tile_pool(name="qtpool", bufs=1) as qtpool,
        tc.tile_pool(name="ps", bufs=1, space="PSUM") as ps,
    ):
        # ---- constants -------------------------------------------------
        vecs = consts.tile([P, 20], F32)   # [nscale|nbias|bq|bk|bproj_eff] x4
        nc.sync.dma_start(out=vecs, in_=t["vecs"][:, :])
        memb = consts.tile([P, 8], F32)    # c -> group-in-tile one-hot
        nc.sync.dma_start(out=memb, in_=t["memb"][:, :])
        membT = consts.tile([8, P], F32)
        nc.sync.dma_start(out=membT, in_=t["membT"][:, :])
        ones_row = consts.tile([1, P], F32)
        nc.vector.memset(ones_row, 1.0)
        ones_col = consts.tile([P, 1], F32)
        nc.vector.memset(ones_col, 1.0)
        A_sb = consts.tile([P, NT], F32)   # per-channel scale (per tile col)
        B_sb = consts.tile([P, NT], F32)   # per-channel shift

        nsc = lambda tt: vecs[:, 0 * NT + tt:0 * NT + tt + 1]
        nbi = lambda tt: vecs[:, 1 * NT + tt:1 * NT + tt + 1]
        bq_ = lambda tt: vecs[:, 2 * NT + tt:2 * NT + tt + 1]
        bk_ = lambda tt: vecs[:, 3 * NT + tt:3 * NT + tt + 1]
        bpe = lambda tt: vecs[:, 4 * NT + tt:4 * NT + tt + 1]

        # ---- phase 1+2: stats, weights, Q^T, K^T, V --------------------
        # x^T is staged once as 4 slabs [128, 4096] in the K^T pool slots via
        # 16 big DMAs (4KB-contiguous rows). Normalized h chunks go into the
        # V-pool slots (slot 4*ch+tt) and are later overwritten by the V tile
        # of the same chunk. No second pass over x from HBM.
        with (
            tc.tile_pool(name="stream", bufs=1) as stream,
            tc.tile_pool(name="wkvpool", bufs=1) as wkvpool,
            tc.tile_pool(name="statsb", bufs=1) as statsb,
        ):
            xslab = [ktpool.tile([P, N], F32, tag=f"kt{tt}", name=f"xs{tt}")
                     for tt in range(NT)]
            for q in range(4):
                for tt in range(NT):
                    nc.sync.dma_start(
                        out=xslab[tt][:, q * 1024:(q + 1) * 1024],
                        in_=t["xT"][tt * P:(tt + 1) * P, q * 1024:(q + 1) * 1024])

            def load_w(dram, idx, eng):
                w = wkvpool.tile([P, C], F32R, tag=f"w{dram.name}{idx}",
                                 name=f"w{dram.name}{idx}")
                eng.dma_start(out=w, in_=dram[idx * P:(idx + 1) * P, :])
                return w

            wq_sb = [load_w(t["wq"], cc, nc.gpsimd) for cc in range(NT)]
            wk_sb = [load_w(t["wk"], cc, nc.sync) for cc in range(NT)]
            wv_sb = [load_w(t["wv"], cc, nc.sync) for cc in range(NT)]

            # pass 1: stats split across DVE (ch 0-4), ACT (ch 5-6), GPS (ch 7)
            NDV = 6
            stats = [statsb.tile([P, NDV, 6], F32, tag=f"st{tt}", name=f"st{tt}")
                     for tt in range(NT)]
            s_extra = statsb.tile([P, NT, 2, 2], F32)   # [tt, unit, (s1, s2)]
            for ch in range(NCH):
                for tt in range(NT):
                    sl = xslab[tt][:, ch * 512:(ch + 1) * 512]
                    if ch >= 2:
                        nc.vector.bn_stats(out=stats[tt][:, ch - 2, :], in_=sl)
                    else:
                        u = ch
                        scr = stream.tile([P, 512], F32, tag="wraw1",
                                          name="ascr", bufs=1)
                        nc.scalar.activation(out=scr, in_=sl, func=AF.Copy,
                                             accum_out=s_extra[:, tt, u, 0:1])
                        scr2 = stream.tile([P, 512], F32, tag="wraw1",
                                           name="ascr2", bufs=1)
                        nc.scalar.activation(out=scr2, in_=sl, func=AF.Square,
                                             accum_out=s_extra[:, tt, u, 1:2])
            mvals = statsb.tile([P, NT, 2], F32)  # mean | E[x^2] per channel
            nsamp_d = float(NDV * 512)
            for tt in range(NT):
                mv = statsb.tile([P, 2], F32, tag="mv", name="mv")
                nc.vector.bn_aggr(out=mv, in_=stats[tt])
                # s1/s2 from the DVE span
                sd = statsb.tile([P, 2], F32, tag="sd", name="sd")
                nc.vector.tensor_scalar_mul(sd[:, 0:1], mv[:, 0:1], nsamp_d)
                msq = statsb.tile([P, 1], F32, tag="msq", name="msq")
                nc.vector.tensor_mul(msq, mv[:, 0:1], mv[:, 0:1])
                nc.vector.tensor_add(sd[:, 1:2], mv[:, 1:2], msq)
                nc.vector.tensor_scalar_mul(sd[:, 1:2], sd[:, 1:2], nsamp_d)
                # add the ACT/GPS partial sums
                tot = statsb.tile([P, 2], F32, tag="tot", name="tot")
                nc.vector.tensor_add(tot, sd, s_extra[:, tt, 0, :])
                nc.vector.tensor_add(tot, tot, s_extra[:, tt, 1, :])
                nc.vector.tensor_scalar_mul(mvals[:, tt, :], tot, 1.0 / 4096.0)
            # group reduction via tiny fp32 matmuls with the membership matrix
            psG = ps.tile([8, 2 * NT], F32, tag="st", name="psG", bufs=2)
            for tt in range(NT):
                nc.tensor.matmul(psG[:, tt:tt + 1], memb, mvals[:, tt, 0:1],
                                 start=True, stop=True)
                nc.tensor.matmul(psG[:, NT + tt:NT + tt + 1], memb,
                                 mvals[:, tt, 1:2], start=True, stop=True)
            MU = statsb.tile([8, NT], F32)
            QQ = statsb.tile([8, NT], F32)
            nc.vector.tensor_scalar_mul(MU, psG[:, 0:NT], 1.0 / 16.0)
            nc.vector.tensor_scalar_mul(QQ, psG[:, NT:2 * NT], 1.0 / 16.0)
            VAR = statsb.tile([8, NT], F32)
            nc.vector.tensor_mul(VAR, MU, MU)
            nc.vector.tensor_sub(VAR, QQ, VAR)
            SD = statsb.tile([8, NT], F32)
            eps_t = statsb.tile([8, 1], F32)
            nc.vector.memset(eps_t, EPS)
            nc.scalar.activation(out=SD, in_=VAR, func=AF.Sqrt, bias=eps_t)
            RSTD = statsb.tile([8, NT], F32)
            nc.vector.reciprocal(RSTD, SD)
            for tt in range(NT):
                psbc = ps.tile([P, 2], F32, tag="st", name="psbc", bufs=2)
                nc.tensor.matmul(psbc[:, 0:1], membT, RSTD[:, tt:tt + 1],
                                 start=True, stop=True)
                nc.tensor.matmul(psbc[:, 1:2], membT, MU[:, tt:tt + 1],
                                 start=True, stop=True)
                nc.vector.tensor_mul(A_sb[:, tt:tt + 1], psbc[:, 0:1], nsc(tt))
                tmp = statsb.tile([P, 1], F32, tag="tmp", name="tmp")
                nc.vector.tensor_mul(tmp, psbc[:, 1:2], A_sb[:, tt:tt + 1])
                nc.vector.tensor_sub(B_sb[:, tt:tt + 1], nbi(tt), tmp)

            # Q^T: load xq^T, normalize, project (+bq), in 512-col halves
            QT_sb = [qtpool.tile([P, NQ], F32, tag=f"qt{o}", name=f"qt{o}")
                     for o in range(NT)]
            for isl in range(NQ // 512):
                hq = []
                for tt in range(NT):
                    xq_t = stream.tile([P, 512], F32, tag=f"xqr{tt}",
                                       name=f"xqr{tt}", bufs=1)
                    nc.gpsimd.dma_start(
                        out=xq_t,
                        in_=t["xqT"][tt * P:(tt + 1) * P, isl * 512:(isl + 1) * 512])
                    hqt = stream.tile([P, 512], F32, tag=f"hq{tt}",
                                      name=f"hq{tt}", bufs=1)
                    nc.vector.tensor_scalar(out=r(hqt), in0=xq_t,
                                            scalar1=A_sb[:, tt:tt + 1],
                                            scalar2=B_sb[:, tt:tt + 1],
                                            op0=OP.mult, op1=OP.add)
                    hq.append(hqt)
                for o in range(NT):
                    pq = ps.tile([P, 512], F32, tag="proj", name="pq", bufs=2)
                    for cc in range(NT):
                        nc.tensor.matmul(
                            pq, r(wq_sb[cc][:, o * P:(o + 1) * P]), r(hq[cc]),
                            start=(cc == 0), stop=(cc == NT - 1))
                    nc.vector.tensor_scalar_add(
                        r(QT_sb[o][:, isl * 512:(isl + 1) * 512]), pq, bq_(o))

            # K^T and V for the LOCAL 1024 tokens (chunks 0-1 of the
            # rotated x), then AllGather across the 4-core replica group.
            KT_sb = [ktpool.tile([P, N], F32R, tag=f"kt{o}", name=f"kt{o}")
                     for o in range(NT)]
            for ch in range(2):
                hch = []
                for tt in range(NT):
                    h = stream.tile([P, 512], F32, tag=f"h{tt}", name=f"h{tt}", bufs=1)
                    if tt < 2:
                        nc.vector.tensor_scalar(
                            out=r(h), in0=xslab[tt][:, ch * 512:(ch + 1) * 512],
                            scalar1=A_sb[:, tt:tt + 1],
                            scalar2=B_sb[:, tt:tt + 1],
                            op0=OP.mult, op1=OP.add)
                    else:
                        nc.scalar.activation(
                            out=r(h), in_=xslab[tt][:, ch * 512:(ch + 1) * 512],
                            func=AF.Identity,
                            bias=B_sb[:, tt:tt + 1],
                            scale=A_sb[:, tt:tt + 1])
                    hch.append(h)
                for o in range(NT):
                    pk = ps.tile([P, 512], F32, tag="proj", name="pk", bufs=2)
                    for cc in range(NT):
                        nc.tensor.matmul(pk, r(wk_sb[cc][:, o * P:(o + 1) * P]),
                                         r(hch[cc]),
                                         start=(cc == 0), stop=(cc == NT - 1))
                    klo = stream.tile([P, 512], F32R, tag=f"hq{2 + o % 2}",
                                      name="klo", bufs=1)
                    nc.vector.tensor_scalar_add(klo, pk, bk_(o))
                    nc.sync.dma_start(
                        out=t["kloc"][o * P:(o + 1) * P, ch * 512:(ch + 1) * 512],
                        in_=klo)
                for nb in range(4):
                    pv = ps.tile([P, C], F32, tag="proj", name="pv", bufs=2)
                    for cc in range(NT):
                        nc.tensor.matmul(pv,
                                         r(hch[cc][:, nb * P:(nb + 1) * P]),
                                         r(wv_sb[cc]),
                                         start=(cc == 0), stop=(cc == NT - 1))
                    i = ch * 4 + nb
                    vlo = stream.tile([P, C], F32R, tag=f"hq{nb % 2}",
                                      name="vlo", bufs=1)
                    nc.scalar.copy(out=vlo, in_=pv)
                    nc.sync.dma_start(out=t["vloc"][i * P:(i + 1) * P, :],
                                      in_=vlo)
            # gather K^T and V across the replica group
            nc.gpsimd.collective_compute(
                "AllGather", mybir.AluOpType.bypass,
                replica_groups=[[0, 1, 2, 3], [4, 5, 6, 7]],
                ins=[t["kloc"][:, :].opt()], outs=[t["kgat"][:, :].opt()])
            nc.gpsimd.collective_compute(
                "AllGather", mybir.AluOpType.bypass,
                replica_groups=[[0, 1, 2, 3], [4, 5, 6, 7]],
                ins=[t["vloc"][:, :].opt()], outs=[t["vgat"][:, :].opt()])
            for src_r in range(4):
                for o in range(NT):
                    nc.sync.dma_start(
                        out=KT_sb[o][:, src_r * NQ:(src_r + 1) * NQ],
                        in_=t["kgat"][src_r * C + o * P:src_r * C + (o + 1) * P, :])
            V_sb = []
            for i in range(NJ):
                vtag = f"v{i}" if i < 28 else f"vs{i - 28}"
                vt = vpool.tile([P, C], F32R, tag=vtag, name=f"v{i}")
                nc.sync.dma_start(out=vt, in_=t["vgat"][i * P:(i + 1) * P, :])
                V_sb.append(vt)

        # ---- phase 3: attention + output projection --------------------
        with (
            tc.tile_pool(name="attnsb", bufs=2) as attnsb,
            tc.tile_pool(name="epool", bufs=2) as epool,
        ):
            wproj_sb = []
            for cc in range(NT):
                w = attnsb.tile([P, C], F32R, tag=f"wp{cc}", name=f"wp{cc}", bufs=1)
                nc.sync.dma_start(out=w, in_=t["wproj"][cc * P:(cc + 1) * P, :])
                wproj_sb.append(w)
            for ih in range(NQ // 512):
                i0 = ih * 512
                # prefetch the residual tiles for this half
                res_t = []
                for o in range(NT):
                    res = attnsb.tile([P, 512], F32, tag=f"res{o}", name=f"res{o}", bufs=1)
                    nc.sync.dma_start(
                        out=res, in_=t["xqT"][o * P:(o + 1) * P, i0:i0 + 512])
                    nc.vector.tensor_scalar_add(res, res, bpe(o))
                    res_t.append(res)
                ps_ot = [ps.tile([P, 512], F32, tag=f"ot{c}", name=f"ot{c}")
                         for c in range(NT)]
                acc = attnsb.tile([P, 512], F32, tag="acc", name="acc")
                for jt in range(NJ):
                    ps_st = ps.tile([P, 512], F32, tag="st", name="st", bufs=2)
                    for o in range(NT):
                        nc.tensor.matmul(
                            ps_st, r(KT_sb[o][:, jt * P:(jt + 1) * P]),
                            r(QT_sb[o][:, i0:i0 + 512]),
                            start=(o == 0), stop=(o == NT - 1))
                    e = epool.tile([P, 512], F32, tag="e", name="e")
                    nc.scalar.activation(out=r(e), in_=ps_st, func=AF.Exp,
                                         scale=SM_SCALE)
                    er = r(e)
                    # denominator partials accumulate on DVE, not PE
                    if jt == 0:
                        nc.vector.tensor_copy(out=acc, in_=e)
                    else:
                        nc.vector.tensor_add(acc, acc, e)
                    first, last = (jt == 0), (jt == NJ - 1)
                    for c in range(NT):
                        nc.tensor.matmul(ps_ot[c],
                                         r(V_sb[jt][:, c * P:(c + 1) * P]),
                                         er, start=first, stop=last)
                # softmax denominator: partition-sum of acc, reciprocal, bcast
                ps_d = ps.tile([1, 512], F32, tag="st", name="psd", bufs=2)
                nc.tensor.matmul(ps_d, ones_col, acc,
                                 start=True, stop=True)
                d_sb = attnsb.tile([1, 512], F32, tag="dsb", name="dsb")
                nc.vector.tensor_copy(out=d_sb, in_=ps_d)
                dr_sb = attnsb.tile([1, 512], F32, tag="drsb", name="drsb")
                nc.vector.reciprocal(dr_sb, d_sb)
                ps_b = ps.tile([P, 512], F32, tag="proj", name="psb", bufs=2)
                nc.tensor.matmul(ps_b, ones_row, dr_sb, start=True, stop=True)
                db_sb = attnsb.tile([P, 512], F32, tag="db", name="db", bufs=1)
                nc.vector.tensor_copy(out=db_sb, in_=ps_b)
                # normalize O^T
                ot_sb = []
                for c in range(NT):
                    o_sb = attnsb.tile([P, 512], F32, tag=f"osb{c}", name=f"osb{c}", bufs=1)
                    nc.vector.tensor_mul(r(o_sb), ps_ot[c], db_sb)
                    ot_sb.append(o_sb)
                # output projection + bias + residual
                for o in range(NT):
                    ps_o = ps.tile([P, 512], F32, tag="proj", name="ps_o", bufs=2)
                    for cc in range(NT):
                        nc.tensor.matmul(ps_o,
                                         r(wproj_sb[cc][:, o * P:(o + 1) * P]),
                                         r(ot_sb[cc]),
                                         start=(cc == 0), stop=(cc == NT - 1))
                    outt = attnsb.tile([P, 512], F32, tag="outt", name="outt")
                    nc.vector.tensor_add(outt, ps_o, res_t[o])
                    nc.sync.dma_start(
                        out=t["outT"][o * P:(o + 1) * P, i0:i0 + 512], in_=outt)


def _build_nc():
    nc = bacc.Bacc("TRN2", target_bir_lowering=False, debug=False)
    dp = nc.declare_dram_parameter
    t = {
        "xT": dp("xT", [C, N], F32, isOutput=False),
        "xqT": dp("xqT", [C, NQ], F32, isOutput=False),
        "wq": dp("wq", [C, C], F32R, isOutput=False),
        "wk": dp("wk", [C, C], F32R, isOutput=False),
        "wv": dp("wv", [C, C], F32R, isOutput=False),
        "wproj": dp("wproj", [C, C], F32R, isOutput=False),
        "vecs": dp("vecs", [P, 20], F32, isOutput=False),
        "memb": dp("memb", [P, 8], F32, isOutput=False),
        "membT": dp("membT", [8, P], F32, isOutput=False),
        "outT": dp("outT", [C, NQ], F32, isOutput=True),
    }
    t["kloc"] = nc.dram_tensor("kloc", [C, NQ], F32R)
    t["vloc"] = nc.dram_tensor("vloc", [NQ, C], F32R)
    t["kgat"] = nc.dram_tensor("kgat", [4 * C, NQ], F32R)
    t["vgat"] = nc.dram_tensor("vgat", [N, C], F32R)
    with tile.TileContext(nc, num_cores=NCORES) as tc:
        _emit(tc, t)
    nc.finalize()
    return nc


def get_nc():
    if "nc" not in _CACHE:
        _CACHE["nc"] = _build_nc()
    return _CACHE["nc"]


def prep_in_maps(x, norm_scale, norm_bias, wq, bq, wk, bk, wv, bv, wproj, bproj):
    f = lambda a: np.ascontiguousarray(np.asarray(a), dtype=np.float32)
    x = f(x)
    wq, wk, wv, wproj = f(wq), f(wk), f(wv), f(wproj)
    bproj_eff = f(bproj) + f(bv) @ wproj
    vecs = np.zeros((P, 20), np.float32)
    for idx, v in enumerate([f(norm_scale), f(norm_bias), f(bq), f(bk), bproj_eff]):
        vecs[:, idx * NT:(idx + 1) * NT] = v.reshape(NT, P).T
    memb = np.zeros((P, 8), np.float32)
    memb[np.arange(P), np.arange(P) // 16] = 1.0
    membT = np.ascontiguousarray(memb.T)
    xr = x.reshape(B, N, C)
    in_maps = []
    xT_cache = {}
    for core in range(NCORES):
        b, qc = divmod(core, 4)
        if b not in xT_cache:
            xT_cache[b] = np.ascontiguousarray(xr[b].T)
        s = qc * NQ
        xTb = xT_cache[b]
        xT_rot = np.ascontiguousarray(np.concatenate([xTb[:, s:], xTb[:, :s]], axis=1))
        xqT = np.ascontiguousarray(xr[b, qc * NQ:(qc + 1) * NQ, :].T)
        in_maps.append({
            "xT": xT_rot, "xqT": xqT, "wq": wq, "wk": wk, "wv": wv,
            "wproj": wproj, "vecs": vecs, "memb": memb, "membT": membT,
        })
    return in_maps


def assemble(results):
    out = np.empty((B, N, C), np.float32)
    for core in range(NCORES):
        b, qc = divmod(core, 4)
        out[b, qc * NQ:(qc + 1) * NQ, :] = results[core]["outT"].T
    return out.reshape(B, 64, 64, C)


def run(trace=False, **inputs):
    nc = get_nc()
    in_maps = prep_in_maps(**inputs)
    res = run_bass_kernel_spmd(nc, in_maps, list(range(NCORES)), trace=trace)
    return assemble(res.results), res


def kernel(**inputs):
    out, _ = run(trace=False, **inputs)
    return out



# revision 6
# speedup vs baseline: 1.9624x; 1.9624x over previous
"""AttnBlock (GroupNorm + single-head 4096-token attention + proj + residual)
on 8 Trainium2 NeuronCores.

Sharding: core = (batch b = core//4, query-chunk qc = core%4).
Each core redundantly computes GroupNorm stats AND the full K/V for its
batch (K/V are needed by every query) directly from the x slab it already
loads for the stats — no collectives, no DRAM roundtrip for K/V.
Attention/proj run for the core's 1024 queries.

All tensors are kept channel-major ("transposed", [C, n]) on chip so that
every matmul is expressible without any on-chip transposes:
  h^T[c,n]  = A[c] * x^T[c,n] + B[c]     (GroupNorm folded to per-channel)
  K^T[o,n]  = wk[c,o].T @ h^T[c,n]       (written in place over the x slab)
  V[n,c]    = h^T[c,n-blk].T @ wv[c,:]
  Q^T[o,i]  = wq[c,o].T @ h^T[c,i]       (i = local 1024 query cols)
  S^T[j,i]  = K^T[o,j-blk].T @ Q^T[o,i]  (accum over 4 o-tiles)
  E = exp(S^T/sqrt(C))  (no max-subtraction; scores are O(1) for this model)
  D[1,i]    = ones[j,1].T @ E            (softmax denominator via PE)
  O^T[c,i]  = V[j,c-blk].T @ E           (accum over all 32 j-tiles in PSUM)
  out^T[o,i]= wproj[c,o].T @ (O^T * (1/D)) + bproj_eff + xq^T
bv is folded into bproj_eff = bproj + bv @ wproj on the host.
Matmuls run as float32r (full PE rate at moving-dim 512, ~fp32 precision).
"""

import os
import sys

import numpy as np

sys.path.insert(0, "/opt/trn_rl_repo")

import concourse.bass as bass
import concourse.bacc as bacc
import concourse.tile as tile
from concourse import mybir
from concourse.bass_utils import run_bass_kernel_spmd

F32 = mybir.dt.float32
F32R = mybir.dt.float32r
AF = mybir.ActivationFunctionType
OP = mybir.AluOpType

B = 2
C = 512
N = 4096          # H*W tokens per batch
NQ = 1024         # queries per core
P = 128
NT = C // P       # 4 channel tiles
NCH = N // 512    # 8 column chunks of x
EPS = 1e-6
SM_SCALE = float(C) ** -0.5
NCORES = 8

_CACHE = {}
USE_CC = False


def _emit(tc, t):
    """Emit the whole per-core kernel. `t` maps name -> DRAM tensor handle."""
    nc = tc.nc
    r = lambda ap: ap.bitcast(F32R)
    NJ = N // P  # 32 j-tiles

    with (
        tc.tile_pool(name="consts", bufs=1) as consts,
        tc.tile_pool(name="ktpool", bufs=1) as ktpool,
        tc.tile_pool(name="vpool", bufs=1) as vpool,
        tc.tile_pool(name="qtpool", bufs=1) as qtpool,
        tc.tile_pool(name="ps", bufs=1, space="PSUM") as ps,
    ):
        # ---- constants -------------------------------------------------
        vecs = consts.tile([P, 20], F32)   # [nscale|nbias|bq|bk|bproj_eff] x4
        nc.sync.dma_start(out=vecs, in_=t["vecs"][:, :])
        memb = consts.tile([P, 8], F32)    # c -> group-in-tile one-hot
        nc.sync.dma_start(out=memb, in_=t["memb"][:, :])
        membT = consts.tile([8, P], F32)
        nc.sync.dma_start(out=membT, in_=t["membT"][:, :])
        ones_row = consts.tile([1, P], F32)
        nc.vector.memset(ones_row, 1.0)
        ones_col = consts.tile([P, 1], F32)
        nc.vector.memset(ones_col, 1.0)
        A_sb = consts.tile([P, NT], F32)   # per-channel scale (per tile col)
        B_sb = consts.tile([P, NT], F32)   # per-channel shift

        nsc = lambda tt: vecs[:, 0 * NT + tt:0 * NT + tt + 1]
        nbi = lambda tt: vecs[:, 1 * NT + tt:1 * NT + tt + 1]
        bq_ = lambda tt: vecs[:, 2 * NT + tt:2 * NT + tt + 1]
        bk_ = lambda tt: vecs[:, 3 * NT + tt:3 * NT + tt + 1]
        bpe = lambda tt: vecs[:, 4 * NT + tt:4 * NT + tt + 1]

        # ---- phase 1+2: stats, weights, Q^T, K^T, V --------------------
        # x^T is staged once as 4 slabs [128, 4096] in the K^T pool slots via
        # 16 big DMAs (4KB-contiguous rows). Each chunk is normalized into a
        # double-buffered h tile and immediately projected: K^T overwrites
        # the slab columns of the chunk it came from, V goes to the V pool,
        # Q^T (chunks 0-1 = the core's own tokens) to the Q pool.
        with (
            tc.tile_pool(name="stream", bufs=1) as stream,
            tc.tile_pool(name="wkvpool", bufs=1) as wkvpool,
            tc.tile_pool(name="statsb", bufs=1) as statsb,
        ):
            xslab = [ktpool.tile([P, N], F32R, tag=f"kt{tt}", name=f"xs{tt}")
                     for tt in range(NT)]
            for q in range(4):
                for tt in range(NT):
                    nc.sync.dma_start(
                        out=xslab[tt][:, q * 1024:(q + 1) * 1024],
                        in_=t["xT"][tt * P:(tt + 1) * P, q * 1024:(q + 1) * 1024])

            def load_w(dram, idx, eng):
                w = wkvpool.tile([P, C], F32R, tag=f"w{dram.name}{idx}",
                                 name=f"w{dram.name}{idx}")
                eng.dma_start(out=w, in_=dram[idx * P:(idx + 1) * P, :])
                return w

            wq_sb = [load_w(t["wq"], cc, nc.gpsimd) for cc in range(NT)]
            wk_sb = [load_w(t["wk"], cc, nc.gpsimd) for cc in range(NT)]
            wv_sb = [load_w(t["wv"], cc, nc.gpsimd) for cc in range(NT)]

            # pass 1: stats split across DVE (ch 2-7) and ACT (ch 0-1)
            NDV = 6
            stats = [statsb.tile([P, NDV, 6], F32, tag=f"st{tt}", name=f"st{tt}")
                     for tt in range(NT)]
            s_extra = statsb.tile([P, NT, 2, 2], F32)   # [tt, unit, (s1, s2)]
            for ch in range(NCH):
                for tt in range(NT):
                    sl = xslab[tt][:, ch * 512:(ch + 1) * 512]
                    if ch >= 2:
                        nc.vector.bn_stats(out=stats[tt][:, ch - 2, :], in_=sl)
                    else:
                        u = ch
                        scr = stream.tile([P, 512], F32, tag="wraw1",
                                          name="ascr", bufs=1)
                        nc.scalar.activation(out=scr, in_=sl, func=AF.Copy,
                                             accum_out=s_extra[:, tt, u, 0:1])
                        scr2 = stream.tile([P, 512], F32, tag="wraw1",
                                           name="ascr2", bufs=1)
                        nc.scalar.activation(out=scr2, in_=sl, func=AF.Square,
                                             accum_out=s_extra[:, tt, u, 1:2])
            mvals = statsb.tile([P, NT, 2], F32)  # mean | E[x^2] per channel
            nsamp_d = float(NDV * 512)
            for tt in range(NT):
                mv = statsb.tile([P, 2], F32, tag="mv", name="mv")
                nc.vector.bn_aggr(out=mv, in_=stats[tt])
                # s1/s2 from the DVE span
                sd = statsb.tile([P, 2], F32, tag="sd", name="sd")
                nc.vector.tensor_scalar_mul(sd[:, 0:1], mv[:, 0:1], nsamp_d)
                msq = statsb.tile([P, 1], F32, tag="msq", name="msq")
                nc.vector.tensor_mul(msq, mv[:, 0:1], mv[:, 0:1])
                nc.vector.tensor_add(sd[:, 1:2], mv[:, 1:2], msq)
                nc.vector.tensor_scalar_mul(sd[:, 1:2], sd[:, 1:2], nsamp_d)
                # add the ACT partial sums
                tot = statsb.tile([P, 2], F32, tag="tot", name="tot")
                nc.vector.tensor_add(tot, sd, s_extra[:, tt, 0, :])
                nc.vector.tensor_add(tot, tot, s_extra[:, tt, 1, :])
                nc.vector.tensor_scalar_mul(mvals[:, tt, :], tot, 1.0 / 4096.0)
            # group reduction via tiny fp32 matmuls with the membership matrix
            psG = ps.tile([8, 2 * NT], F32, tag="st", name="psG", bufs=2)
            for tt in range(NT):
                nc.tensor.matmul(psG[:, tt:tt + 1], memb, mvals[:, tt, 0:1],
                                 start=True, stop=True)
                nc.tensor.matmul(psG[:, NT + tt:NT + tt + 1], memb,
                                 mvals[:, tt, 1:2], start=True, stop=True)
            MU = statsb.tile([8, NT], F32)
            QQ = statsb.tile([8, NT], F32)
            nc.vector.tensor_scalar_mul(MU, psG[:, 0:NT], 1.0 / 16.0)
            nc.vector.tensor_scalar_mul(QQ, psG[:, NT:2 * NT], 1.0 / 16.0)
            VAR = statsb.tile([8, NT], F32)
            nc.vector.tensor_mul(VAR, MU, MU)
            nc.vector.tensor_sub(VAR, QQ, VAR)
            SD = statsb.tile([8, NT], F32)
            eps_t = statsb.tile([8, 1], F32)
            nc.vector.memset(eps_t, EPS)
            nc.scalar.activation(out=SD, in_=VAR, func=AF.Sqrt, bias=eps_t)
            RSTD = statsb.tile([8, NT], F32)
            nc.vector.reciprocal(RSTD, SD)
            for tt in range(NT):
                psbc = ps.tile([P, 2], F32, tag="st", name="psbc", bufs=2)
                nc.tensor.matmul(psbc[:, 0:1], membT, RSTD[:, tt:tt + 1],
                                 start=True, stop=True)
                nc.tensor.matmul(psbc[:, 1:2], membT, MU[:, tt:tt + 1],
                                 start=True, stop=True)
                nc.vector.tensor_mul(A_sb[:, tt:tt + 1], psbc[:, 0:1], nsc(tt))
                tmp = statsb.tile([P, 1], F32, tag="tmp", name="tmp")
                nc.vector.tensor_mul(tmp, psbc[:, 1:2], A_sb[:, tt:tt + 1])
                nc.vector.tensor_sub(B_sb[:, tt:tt + 1], nbi(tt), tmp)

            # pass 2: per 512-token chunk: normalize, project K/V (+Q for
            # the local chunks 0-1). K^T overwrites the slab columns.
            QT_sb = [qtpool.tile([P, NQ], F32, tag=f"qt{o}", name=f"qt{o}")
                     for o in range(NT)]
            V_sb = []
            for i in range(NJ):
                vt = vpool.tile([P, C], F32R, tag=f"v{i}", name=f"v{i}")
                V_sb.append(vt)
            for ch in range(NCH):
                hch = []
                for tt in range(NT):
                    h = stream.tile([P, 512], F32, tag=f"h{tt}_{ch % 2}",
                                    name=f"h{tt}", bufs=1)
                    if tt < 2:
                        nc.vector.tensor_scalar(
                            out=r(h), in0=xslab[tt][:, ch * 512:(ch + 1) * 512],
                            scalar1=A_sb[:, tt:tt + 1],
                            scalar2=B_sb[:, tt:tt + 1],
                            op0=OP.mult, op1=OP.add)
                    else:
                        nc.scalar.activation(
                            out=r(h), in_=xslab[tt][:, ch * 512:(ch + 1) * 512],
                            func=AF.Identity,
                            bias=B_sb[:, tt:tt + 1],
                            scale=A_sb[:, tt:tt + 1])
                    hch.append(h)
                # K^T for this chunk -> slab columns (x already consumed)
                for o in range(NT):
                    pk = ps.tile([P, 512], F32, tag=f"proj{o % 2}",
                                 name="pk", bufs=1)
                    for cc in range(NT):
                        nc.tensor.matmul(pk, r(wk_sb[cc][:, o * P:(o + 1) * P]),
                                         r(hch[cc]),
                                         start=(cc == 0), stop=(cc == NT - 1))
                    dst = xslab[o][:, ch * 512:(ch + 1) * 512]
                    if o < 2:
                        nc.vector.tensor_scalar_add(dst, pk, bk_(o))
                    else:
                        nc.scalar.activation(out=dst, in_=pk,
                                             func=AF.Identity, bias=bk_(o))
                # V for this chunk -> V pool
                for nb in range(4):
                    pv = ps.tile([P, C], F32, tag=f"proj{nb % 2}",
                                 name="pv", bufs=1)
                    for cc in range(NT):
                        nc.tensor.matmul(pv,
                                         r(hch[cc][:, nb * P:(nb + 1) * P]),
                                         r(wv_sb[cc]),
                                         start=(cc == 0), stop=(cc == NT - 1))
                    if nb < 2:
                        nc.vector.tensor_copy(out=V_sb[ch * 4 + nb], in_=pv)
                    else:
                        nc.scalar.copy(out=V_sb[ch * 4 + nb], in_=pv)
                # Q^T for the local chunks
                if ch < 2:
                    for o in range(NT):
                        pq = ps.tile([P, 512], F32, tag=f"proj{o % 2}",
                                     name="pq", bufs=1)
                        for cc in range(NT):
                            nc.tensor.matmul(
                                pq, r(wq_sb[cc][:, o * P:(o + 1) * P]),
                                r(hch[cc]),
                                start=(cc == 0), stop=(cc == NT - 1))
                        dst = r(QT_sb[o][:, ch * 512:(ch + 1) * 512])
                        if o < 2:
                            nc.vector.tensor_scalar_add(dst, pq, bq_(o))
                        else:
                            nc.scalar.activation(out=dst, in_=pq,
                                                 func=AF.Identity, bias=bq_(o))
            KT_sb = xslab  # the slab now holds K^T[o, 0:4096]

        # ---- phase 3: attention + output projection --------------------
        with (
            tc.tile_pool(name="attnsb", bufs=2) as attnsb,
            tc.tile_pool(name="epool", bufs=2) as epool,
        ):
            wproj_sb = []
            for cc in range(NT):
                w = attnsb.tile([P, C], F32R, tag=f"wp{cc}", name=f"wp{cc}", bufs=1)
                nc.sync.dma_start(out=w, in_=t["wproj"][cc * P:(cc + 1) * P, :])
                wproj_sb.append(w)
            for ih in range(NQ // 512):
                i0 = ih * 512
                # prefetch the residual tiles for this half
                res_t = []
                for o in range(NT):
                    res = attnsb.tile([P, 512], F32, tag=f"res{o}", name=f"res{o}", bufs=1)
                    nc.sync.dma_start(
                        out=res, in_=t["xqT"][o * P:(o + 1) * P, i0:i0 + 512])
                    nc.vector.tensor_scalar_add(res, res, bpe(o))
                    res_t.append(res)
                ps_ot = [ps.tile([P, 512], F32, tag=f"ot{c}", name=f"ot{c}")
                         for c in range(NT)]
                acc = attnsb.tile([P, 512], F32, tag="acc", name="acc")
                for jt in range(NJ):
                    ps_st = ps.tile([P, 512], F32, tag="st", name="st", bufs=2)
                    for o in range(NT):
                        nc.tensor.matmul(
                            ps_st, r(KT_sb[o][:, jt * P:(jt + 1) * P]),
                            r(QT_sb[o][:, i0:i0 + 512]),
                            start=(o == 0), stop=(o == NT - 1))
                    e = epool.tile([P, 512], F32, tag="e", name="e")
                    nc.scalar.activation(out=r(e), in_=ps_st, func=AF.Exp,
                                         scale=SM_SCALE)
                    er = r(e)
                    # denominator partials accumulate on DVE, not PE
                    if jt == 0:
                        nc.vector.tensor_copy(out=acc, in_=e)
                    else:
                        nc.vector.tensor_add(acc, acc, e)
                    first, last = (jt == 0), (jt == NJ - 1)
                    for c in range(NT):
                        nc.tensor.matmul(ps_ot[c],
                                         r(V_sb[jt][:, c * P:(c + 1) * P]),
                                         er, start=first, stop=last)
                # softmax denominator: partition-sum of acc, reciprocal, bcast
                ps_d = ps.tile([1, 512], F32, tag="st", name="psd", bufs=2)
                nc.tensor.matmul(ps_d, ones_col, acc,
                                 start=True, stop=True)
                d_sb = attnsb.tile([1, 512], F32, tag="dsb", name="dsb")
                nc.vector.tensor_copy(out=d_sb, in_=ps_d)
                dr_sb = attnsb.tile([1, 512], F32, tag="drsb", name="drsb")
                nc.vector.reciprocal(dr_sb, d_sb)
                ps_b = ps.tile([P, 512], F32, tag="proj0", name="psb", bufs=1)
                nc.tensor.matmul(ps_b, ones_row, dr_sb, start=True, stop=True)
                db_sb = attnsb.tile([P, 512], F32, tag="db", name="db", bufs=1)
                nc.vector.tensor_copy(out=db_sb, in_=ps_b)
                # normalize O^T
                ot_sb = []
                for c in range(NT):
                    o_sb = attnsb.tile([P, 512], F32, tag=f"osb{c}", name=f"osb{c}", bufs=1)
                    nc.vector.tensor_mul(r(o_sb), ps_ot[c], db_sb)
                    ot_sb.append(o_sb)
                # output projection + bias + residual
                for o in range(NT):
                    ps_o = ps.tile([P, 512], F32, tag=f"proj{o % 2}",
                                   name="ps_o", bufs=1)
                    for cc in range(NT):
                        nc.tensor.matmul(ps_o,
                                         r(wproj_sb[cc][:, o * P:(o + 1) * P]),
                                         r(ot_sb[cc]),
                                         start=(cc == 0), stop=(cc == NT - 1))
                    outt = attnsb.tile([P, 512], F32, tag="outt", name="outt")
                    nc.vector.tensor_add(outt, ps_o, res_t[o])
                    nc.sync.dma_start(
                        out=t["outT"][o * P:(o + 1) * P, i0:i0 + 512], in_=outt)


def _build_nc():
    nc = bacc.Bacc("TRN2", target_bir_lowering=False, debug=False)
    dp = nc.declare_dram_parameter
    t = {
        "xT": dp("xT", [C, N], F32R, isOutput=False),
        "xqT": dp("xqT", [C, NQ], F32, isOutput=False),
        "wq": dp("wq", [C, C], F32R, isOutput=False),
        "wk": dp("wk", [C, C], F32R, isOutput=False),
        "wv": dp("wv", [C, C], F32R, isOutput=False),
        "wproj": dp("wproj", [C, C], F32R, isOutput=False),
        "vecs": dp("vecs", [P, 20], F32, isOutput=False),
        "memb": dp("memb", [P, 8], F32, isOutput=False),
        "membT": dp("membT", [8, P], F32, isOutput=False),
        "outT": dp("outT", [C, NQ], F32, isOutput=True),
    }
    with tile.TileContext(nc, num_cores=NCORES) as tc:
        _emit(tc, t)
    nc.finalize()
    return nc


def get_nc():
    if "nc" not in _CACHE:
        _CACHE["nc"] = _build_nc()
    return _CACHE["nc"]


def prep_in_maps(x, norm_scale, norm_bias, wq, bq, wk, bk, wv, bv, wproj, bproj):
    f = lambda a: np.ascontiguousarray(np.asarray(a), dtype=np.float32)
    x = f(x)
    wq, wk, wv, wproj = f(wq), f(wk), f(wv), f(wproj)
    bproj_eff = f(bproj) + f(bv) @ wproj
    vecs = np.zeros((P, 20), np.float32)
    for idx, v in enumerate([f(norm_scale), f(norm_bias), f(bq), f(bk), bproj_eff]):
        vecs[:, idx * NT:(idx + 1) * NT] = v.reshape(NT, P).T
    memb = np.zeros((P, 8), np.float32)
    memb[np.arange(P), np.arange(P) // 16] = 1.0
    membT = np.ascontiguousarray(memb.T)
    xr = x.reshape(B, N, C)
    in_maps = []
    xT_cache = {}
    for core in range(NCORES):
        b, qc = divmod(core, 4)
        if b not in xT_cache:
            xT_cache[b] = np.ascontiguousarray(xr[b].T)
        s = qc * NQ
        xTb = xT_cache[b]
        xT_rot = np.ascontiguousarray(np.concatenate([xTb[:, s:], xTb[:, :s]], axis=1))
        xqT = np.ascontiguousarray(xr[b, qc * NQ:(qc + 1) * NQ, :].T)
        in_maps.append({
            "xT": xT_rot, "xqT": xqT, "wq": wq, "wk": wk, "wv": wv,
            "wproj": wproj, "vecs": vecs, "memb": memb, "membT": membT,
        })
    return in_maps


def assemble(results):
    out = np.empty((B, N, C), np.float32)
    for core in range(NCORES):
        b, qc = divmod(core, 4)
        out[b, qc * NQ:(qc + 1) * NQ, :] = results[core]["outT"].T
    return out.reshape(B, 64, 64, C)


def run(trace=False, **inputs):
    nc = get_nc()
    in_maps = prep_in_maps(**inputs)
    res = run_bass_kernel_spmd(nc, in_maps, list(range(NCORES)), trace=trace)
    return assemble(res.results), res


def kernel(**inputs):
    out, _ = run(trace=False, **inputs)
    return out


# revision 16
# speedup vs baseline: 2.2629x; 1.1531x over previous
"""AttnBlock (GroupNorm + single-head 4096-token attention + proj + residual)
on 8 Trainium2 NeuronCores.

Sharding: core = (batch b = core//4, query-chunk qc = core%4).
Each core redundantly computes GroupNorm stats AND the full K/V for its
batch (K/V are needed by every query) directly from the x slab it already
loads for the stats — no collectives, no DRAM roundtrip for K/V.
Attention/proj run for the core's 1024 queries.

All tensors are kept channel-major ("transposed", [C, n]) on chip so that
every matmul is expressible without any on-chip transposes:
  h^T[c,n]  = A[c] * x^T[c,n] + B[c]     (GroupNorm folded to per-channel)
  K^T[o,n]  = wk[c,o].T @ h^T[c,n]       (written in place over the x slab)
  V[n,c]    = h^T[c,n-blk].T @ wv[c,:]
  Q^T[o,i]  = wq[c,o].T @ h^T[c,i]       (i = local 1024 query cols)
  S^T[j,i]  = K^T[o,j-blk].T @ Q^T[o,i]  (accum over 4 o-tiles)
  E = exp(S^T/sqrt(C))  (no max-subtraction; scores are O(1) for this model)
  D[1,i]    = ones[j,1].T @ E            (softmax denominator via PE)
  O^T[c,i]  = V[j,c-blk].T @ E           (accum over all 32 j-tiles in PSUM)
  out^T[o,i]= wproj[c,o].T @ (O^T * (1/D)) + bproj_eff + xq^T
bv is folded into bproj_eff = bproj + bv @ wproj on the host.
Matmuls run as float32r (full PE rate at moving-dim 512, ~fp32 precision).
"""

import os
import sys

import numpy as np

sys.path.insert(0, "/opt/trn_rl_repo")

import concourse.bass as bass
import concourse.bacc as bacc
import concourse.tile as tile
from concourse import mybir
from concourse.bass_utils import run_bass_kernel_spmd

F32 = mybir.dt.float32
F32R = mybir.dt.float32r
FP8 = mybir.dt.float8e4
DR = mybir.MatmulPerfMode.DoubleRow
AF = mybir.ActivationFunctionType
OP = mybir.AluOpType

B = 2
C = 512
N = 4096          # H*W tokens per batch
NQ = 1024         # queries per core
P = 128
NT = C // P       # 4 channel tiles
NCH = N // 512    # 8 column chunks of x
EPS = 1e-6
SM_SCALE = float(C) ** -0.5
NCORES = 8

_CACHE = {}
USE_CC = False


def _emit(tc, t):
    """Emit the whole per-core kernel. `t` maps name -> DRAM tensor handle."""
    nc = tc.nc
    r = lambda ap: ap.bitcast(F32R)
    NJ = N // P  # 32 j-tiles

    with (
        tc.tile_pool(name="consts", bufs=1) as consts,
        tc.tile_pool(name="ktpool", bufs=1) as ktpool,
        tc.tile_pool(name="vpool", bufs=1) as vpool,
        tc.tile_pool(name="qtpool", bufs=1) as qtpool,
        tc.tile_pool(name="ps", bufs=1, space="PSUM") as ps,
    ):
        # ---- constants -------------------------------------------------
        vecs = consts.tile([P, 20], F32)   # [nscale|nbias|bq|bk|bproj_eff] x4
        nc.sync.dma_start(out=vecs, in_=t["vecs"][:, :])
        memb = consts.tile([P, 8], F32)    # c -> group-in-tile one-hot
        nc.sync.dma_start(out=memb, in_=t["memb"][:, :])
        membT = consts.tile([8, P], F32)
        nc.sync.dma_start(out=membT, in_=t["membT"][:, :])
        ones_row = consts.tile([1, P], F32)
        nc.vector.memset(ones_row, 1.0)
        # pair stride of DoubleRow weights must be 16B-aligned, so pad
        ones8_t = consts.tile([P, 2, 16], FP8)
        nc.vector.memset(ones8_t, 1.0)
        ones8 = ones8_t[:, :, 0:1]
        eshift = consts.tile([P, 1], F32)   # exp shift, see phase 3
        nc.vector.memset(eshift, -2.0)
        A_sb = consts.tile([P, NT], F32)   # per-channel scale (per tile col)
        B_sb = consts.tile([P, NT], F32)   # per-channel shift

        nsc = lambda tt: vecs[:, 0 * NT + tt:0 * NT + tt + 1]
        nbi = lambda tt: vecs[:, 1 * NT + tt:1 * NT + tt + 1]
        bq_ = lambda tt: vecs[:, 2 * NT + tt:2 * NT + tt + 1]
        bk_ = lambda tt: vecs[:, 3 * NT + tt:3 * NT + tt + 1]
        bpe = lambda tt: vecs[:, 4 * NT + tt:4 * NT + tt + 1]

        # ---- phase 1+2: stats, weights, Q^T, K^T, V --------------------
        # x^T is staged once as 4 slabs [128, 4096] in the K^T pool slots via
        # 16 big DMAs (4KB-contiguous rows). Each chunk is normalized into a
        # double-buffered h tile and immediately projected: K^T overwrites
        # the slab columns of the chunk it came from, V goes to the V pool,
        # Q^T (chunks 0-1 = the core's own tokens) to the Q pool.
        with (
            tc.tile_pool(name="stream", bufs=1) as stream,
            tc.tile_pool(name="wkvpool", bufs=1) as wkvpool,
            tc.tile_pool(name="statsb", bufs=1) as statsb,
        ):
            xslab = [ktpool.tile([P, N], F32R, tag=f"kt{tt}", name=f"xs{tt}")
                     for tt in range(NT)]
            for q in range(4):
                for tt in range(NT):
                    nc.sync.dma_start(
                        out=xslab[tt][:, q * 1024:(q + 1) * 1024],
                        in_=t["xT"][tt * P:(tt + 1) * P, q * 1024:(q + 1) * 1024])

            def load_w(dram, idx, eng):
                w = wkvpool.tile([P, C], F32R, tag=f"w{dram.name}{idx}",
                                 name=f"w{dram.name}{idx}")
                eng.dma_start(out=w, in_=dram[idx * P:(idx + 1) * P, :])
                return w

            wq_sb = [load_w(t["wq"], cc, nc.gpsimd) for cc in range(NT)]
            wk_sb = [load_w(t["wk"], cc, nc.gpsimd) for cc in range(NT)]
            wv_sb = [load_w(t["wv"], cc, nc.gpsimd) for cc in range(NT)]

            # pass 1: stats split across DVE (ch 2-7) and ACT (ch 0-1)
            NDV = 6
            stats = [statsb.tile([P, NDV, 6], F32, tag=f"st{tt}", name=f"st{tt}")
                     for tt in range(NT)]
            s_extra = statsb.tile([P, NT, 2, 2], F32)   # [tt, unit, (s1, s2)]
            for ch in range(NCH):
                for tt in range(NT):
                    sl = xslab[tt][:, ch * 512:(ch + 1) * 512]
                    if ch >= 2:
                        nc.vector.bn_stats(out=stats[tt][:, ch - 2, :], in_=sl)
                    else:
                        u = ch
                        scr = stream.tile([P, 512], F32, tag="wraw1",
                                          name="ascr", bufs=1)
                        nc.scalar.activation(out=scr, in_=sl, func=AF.Copy,
                                             accum_out=s_extra[:, tt, u, 0:1])
                        scr2 = stream.tile([P, 512], F32, tag="wraw1",
                                           name="ascr2", bufs=1)
                        nc.scalar.activation(out=scr2, in_=sl, func=AF.Square,
                                             accum_out=s_extra[:, tt, u, 1:2])
            mvals = statsb.tile([P, NT, 2], F32)  # mean | E[x^2] per channel
            nsamp_d = float(NDV * 512)
            for tt in range(NT):
                mv = statsb.tile([P, 2], F32, tag="mv", name="mv")
                nc.vector.bn_aggr(out=mv, in_=stats[tt])
                # s1/s2 from the DVE span
                sd = statsb.tile([P, 2], F32, tag="sd", name="sd")
                nc.vector.tensor_scalar_mul(sd[:, 0:1], mv[:, 0:1], nsamp_d)
                msq = statsb.tile([P, 1], F32, tag="msq", name="msq")
                nc.vector.tensor_mul(msq, mv[:, 0:1], mv[:, 0:1])
                nc.vector.tensor_add(sd[:, 1:2], mv[:, 1:2], msq)
                nc.vector.tensor_scalar_mul(sd[:, 1:2], sd[:, 1:2], nsamp_d)
                # add the ACT partial sums
                tot = statsb.tile([P, 2], F32, tag="tot", name="tot")
                nc.vector.tensor_add(tot, sd, s_extra[:, tt, 0, :])
                nc.vector.tensor_add(tot, tot, s_extra[:, tt, 1, :])
                nc.vector.tensor_scalar_mul(mvals[:, tt, :], tot, 1.0 / 4096.0)
            # group reduction via tiny fp32 matmuls with the membership matrix
            psG = ps.tile([8, 2 * NT], F32, tag="st", name="psG", bufs=2)
            for tt in range(NT):
                nc.tensor.matmul(psG[:, tt:tt + 1], memb, mvals[:, tt, 0:1],
                                 start=True, stop=True)
                nc.tensor.matmul(psG[:, NT + tt:NT + tt + 1], memb,
                                 mvals[:, tt, 1:2], start=True, stop=True)
            MU = statsb.tile([8, NT], F32)
            QQ = statsb.tile([8, NT], F32)
            nc.vector.tensor_scalar_mul(MU, psG[:, 0:NT], 1.0 / 16.0)
            nc.vector.tensor_scalar_mul(QQ, psG[:, NT:2 * NT], 1.0 / 16.0)
            VAR = statsb.tile([8, NT], F32)
            nc.vector.tensor_mul(VAR, MU, MU)
            nc.vector.tensor_sub(VAR, QQ, VAR)
            SD = statsb.tile([8, NT], F32)
            eps_t = statsb.tile([8, 1], F32)
            nc.vector.memset(eps_t, EPS)
            nc.scalar.activation(out=SD, in_=VAR, func=AF.Sqrt, bias=eps_t)
            RSTD = statsb.tile([8, NT], F32)
            nc.vector.reciprocal(RSTD, SD)
            for tt in range(NT):
                psbc = ps.tile([P, 2], F32, tag="st", name="psbc", bufs=2)
                nc.tensor.matmul(psbc[:, 0:1], membT, RSTD[:, tt:tt + 1],
                                 start=True, stop=True)
                nc.tensor.matmul(psbc[:, 1:2], membT, MU[:, tt:tt + 1],
                                 start=True, stop=True)
                nc.vector.tensor_mul(A_sb[:, tt:tt + 1], psbc[:, 0:1], nsc(tt))
                tmp = statsb.tile([P, 1], F32, tag="tmp", name="tmp")
                nc.vector.tensor_mul(tmp, psbc[:, 1:2], A_sb[:, tt:tt + 1])
                nc.vector.tensor_sub(B_sb[:, tt:tt + 1], nbi(tt), tmp)

            # pass 2: per 512-token chunk: normalize, project K/V (+Q for
            # the local chunks 0-1). K^T/Q^T/V quantize to fp8e4m3 on the
            # PSUM->SBUF copy, packed [P, pair, free] for DoubleRow matmuls.
            QT8 = qtpool.tile([P, NT, NQ], FP8, name="qt8")
            KT8 = ktpool.tile([P, NT, N], FP8, tag="kt8", name="kt8")
            V8 = [vpool.tile([P, 2, C], FP8, tag=f"v{i}", name=f"v{i}")
                  for i in range(NJ // 2)]
            for ch in range(NCH):
                hch = []
                for tt in range(NT):
                    h = stream.tile([P, 512], F32, tag=f"h{tt}_{ch % 2}",
                                    name=f"h{tt}", bufs=1)
                    if tt < 2:
                        nc.vector.tensor_scalar(
                            out=r(h), in0=xslab[tt][:, ch * 512:(ch + 1) * 512],
                            scalar1=A_sb[:, tt:tt + 1],
                            scalar2=B_sb[:, tt:tt + 1],
                            op0=OP.mult, op1=OP.add)
                    else:
                        nc.scalar.activation(
                            out=r(h), in_=xslab[tt][:, ch * 512:(ch + 1) * 512],
                            func=AF.Identity,
                            bias=B_sb[:, tt:tt + 1],
                            scale=A_sb[:, tt:tt + 1])
                    hch.append(h)
                # K^T for this chunk -> fp8 pool
                for o in range(NT):
                    pk = ps.tile([P, 512], F32, tag=f"proj{o % 2}",
                                 name="pk", bufs=1)
                    for cc in range(NT):
                        nc.tensor.matmul(pk, r(wk_sb[cc][:, o * P:(o + 1) * P]),
                                         r(hch[cc]),
                                         start=(cc == 0), stop=(cc == NT - 1))
                    dst = KT8[:, o, ch * 512:(ch + 1) * 512]
                    if o < 2:
                        nc.vector.tensor_scalar_add(dst, pk, bk_(o))
                    else:
                        nc.scalar.activation(out=dst, in_=pk,
                                             func=AF.Identity, bias=bk_(o))
                # V for this chunk -> V pool
                for nb in range(4):
                    pv = ps.tile([P, C], F32, tag=f"proj{nb % 2}",
                                 name="pv", bufs=1)
                    for cc in range(NT):
                        nc.tensor.matmul(pv,
                                         r(hch[cc][:, nb * P:(nb + 1) * P]),
                                         r(wv_sb[cc]),
                                         start=(cc == 0), stop=(cc == NT - 1))
                    dst = V8[ch * 2 + nb // 2][:, nb % 2, :]
                    if nb < 2:
                        nc.vector.tensor_copy(out=dst, in_=pv)
                    else:
                        nc.scalar.copy(out=dst, in_=pv)
                # Q^T for the local chunks
                if ch < 2:
                    for o in range(NT):
                        pq = ps.tile([P, 512], F32, tag=f"proj{o % 2}",
                                     name="pq", bufs=1)
                        for cc in range(NT):
                            nc.tensor.matmul(
                                pq, r(wq_sb[cc][:, o * P:(o + 1) * P]),
                                r(hch[cc]),
                                start=(cc == 0), stop=(cc == NT - 1))
                        dst = QT8[:, o, ch * 512:(ch + 1) * 512]
                        if o < 2:
                            nc.vector.tensor_scalar_add(dst, pq, bq_(o))
                        else:
                            nc.scalar.activation(out=dst, in_=pq,
                                                 func=AF.Identity, bias=bq_(o))

        # ---- phase 3: attention + output projection --------------------
        with (
            tc.tile_pool(name="attnsb", bufs=2) as attnsb,
            tc.tile_pool(name="epool", bufs=2) as epool,
        ):
            wproj_sb = []
            for cc in range(NT):
                w = attnsb.tile([P, C], F32R, tag=f"wp{cc}", name=f"wp{cc}", bufs=1)
                nc.sync.dma_start(out=w, in_=t["wproj"][cc * P:(cc + 1) * P, :])
                wproj_sb.append(w)
            for ih in range(NQ // 512):
                i0 = ih * 512
                # residual tiles for this half, straight from the x slab
                res_t = []
                for o in range(NT):
                    res = attnsb.tile([P, 512], F32, tag=f"res{o}", name=f"res{o}", bufs=1)
                    nc.vector.tensor_scalar_add(
                        res, xslab[o][:, i0:i0 + 512], bpe(o))
                    res_t.append(res)
                ps_ot = [ps.tile([P, 512], F32, tag=f"ot{c}", name=f"ot{c}")
                         for c in range(NT)]
                # softmax denominator accumulates on the PE alongside O^T
                ps_d = ps.tile([1, 512], F32, tag="proj0", name="psd", bufs=1)
                for pr in range(NJ // 2):
                    e8 = epool.tile([P, 2, 512], FP8, tag="e", name="e")
                    for half in range(2):
                        jt = pr * 2 + half
                        ps_st = ps.tile([P, 512], F32, tag="st", name="st", bufs=2)
                        for op in range(2):
                            nc.tensor.matmul(
                                ps_st,
                                KT8[:, 2 * op:2 * op + 2, jt * P:(jt + 1) * P],
                                QT8[:, 2 * op:2 * op + 2, i0:i0 + 512],
                                start=(op == 0), stop=(op == 1), perf_mode=DR)
                        # constant -2 shift keeps exp within fp8e4m3 range
                        # (logits are O(1..7)); softmax is shift-invariant
                        nc.scalar.activation(out=e8[:, half, :], in_=ps_st,
                                             func=AF.Exp, scale=SM_SCALE,
                                             bias=eshift)
                    first, last = (pr == 0), (pr == NJ // 2 - 1)
                    for c in range(NT):
                        nc.tensor.matmul(ps_ot[c],
                                         V8[pr][:, :, c * P:(c + 1) * P],
                                         e8, start=first, stop=last,
                                         perf_mode=DR)
                    nc.tensor.matmul(ps_d, ones8, e8, start=first, stop=last,
                                     perf_mode=DR)
                # softmax denominator: reciprocal, bcast
                d_sb = attnsb.tile([1, 512], F32, tag="dsb", name="dsb")
                nc.vector.tensor_copy(out=d_sb, in_=ps_d)
                dr_sb = attnsb.tile([1, 512], F32, tag="drsb", name="drsb")
                nc.vector.reciprocal(dr_sb, d_sb)
                ps_b = ps.tile([P, 512], F32, tag="proj1", name="psb", bufs=1)
                nc.tensor.matmul(ps_b, ones_row, dr_sb, start=True, stop=True)
                db_sb = attnsb.tile([P, 512], F32, tag="db", name="db", bufs=1)
                nc.vector.tensor_copy(out=db_sb, in_=ps_b)
                # normalize O^T
                ot_sb = []
                for c in range(NT):
                    o_sb = attnsb.tile([P, 512], F32, tag=f"osb{c}", name=f"osb{c}", bufs=1)
                    nc.vector.tensor_mul(r(o_sb), ps_ot[c], db_sb)
                    ot_sb.append(o_sb)
                # output projection + bias + residual
                for o in range(NT):
                    ps_o = ps.tile([P, 512], F32, tag=f"proj{o % 2}",
                                   name="ps_o", bufs=1)
                    for cc in range(NT):
                        nc.tensor.matmul(ps_o,
                                         r(wproj_sb[cc][:, o * P:(o + 1) * P]),
                                         r(ot_sb[cc]),
                                         start=(cc == 0), stop=(cc == NT - 1))
                    outt = attnsb.tile([P, 512], F32, tag="outt", name="outt")
                    nc.vector.tensor_add(outt, ps_o, res_t[o])
                    nc.sync.dma_start(
                        out=t["outT"][o * P:(o + 1) * P, i0:i0 + 512], in_=outt)


def _build_nc():
    nc = bacc.Bacc("TRN2", target_bir_lowering=False, debug=False)
    dp = nc.declare_dram_parameter
    t = {
        "xT": dp("xT", [C, N], F32R, isOutput=False),
        "wq": dp("wq", [C, C], F32R, isOutput=False),
        "wk": dp("wk", [C, C], F32R, isOutput=False),
        "wv": dp("wv", [C, C], F32R, isOutput=False),
        "wproj": dp("wproj", [C, C], F32R, isOutput=False),
        "vecs": dp("vecs", [P, 20], F32, isOutput=False),
        "memb": dp("memb", [P, 8], F32, isOutput=False),
        "membT": dp("membT", [8, P], F32, isOutput=False),
        "outT": dp("outT", [C, NQ], F32, isOutput=True),
    }
    with tile.TileContext(nc, num_cores=NCORES) as tc:
        _emit(tc, t)
    nc.finalize()
    return nc


def get_nc():
    if "nc" not in _CACHE:
        _CACHE["nc"] = _build_nc()
    return _CACHE["nc"]


def prep_in_maps(x, norm_scale, norm_bias, wq, bq, wk, bk, wv, bv, wproj, bproj):
    f = lambda a: np.ascontiguousarray(np.asarray(a), dtype=np.float32)
    x = f(x)
    wq, wk, wv, wproj = f(wq), f(wk), f(wv), f(wproj)
    bproj_eff = f(bproj) + f(bv) @ wproj
    vecs = np.zeros((P, 20), np.float32)
    for idx, v in enumerate([f(norm_scale), f(norm_bias), f(bq), f(bk), bproj_eff]):
        vecs[:, idx * NT:(idx + 1) * NT] = v.reshape(NT, P).T
    memb = np.zeros((P, 8), np.float32)
    memb[np.arange(P), np.arange(P) // 16] = 1.0
    membT = np.ascontiguousarray(memb.T)
    xr = x.reshape(B, N, C)
    in_maps = []
    xT_cache = {}
    for core in range(NCORES):
        b, qc = divmod(core, 4)
        if b not in xT_cache:
            xT_cache[b] = np.ascontiguousarray(xr[b].T)
        s = qc * NQ
        xTb = xT_cache[b]
        xT_rot = np.ascontiguousarray(np.concatenate([xTb[:, s:], xTb[:, :s]], axis=1))
        in_maps.append({
            "xT": xT_rot, "wq": wq, "wk": wk, "wv": wv,
            "wproj": wproj, "vecs": vecs, "memb": memb, "membT": membT,
        })
    return in_maps


def assemble(results):
    out = np.empty((B, N, C), np.float32)
    for core in range(NCORES):
        b, qc = divmod(core, 4)
        out[b, qc * NQ:(qc + 1) * NQ, :] = results[core]["outT"].T
    return out.reshape(B, 64, 64, C)


def run(trace=False, **inputs):
    nc = get_nc()
    in_maps = prep_in_maps(**inputs)
    res = run_bass_kernel_spmd(nc, in_maps, list(range(NCORES)), trace=trace)
    return assemble(res.results), res


def kernel(**inputs):
    out, _ = run(trace=False, **inputs)
    return out


# revision 17
# speedup vs baseline: 2.8203x; 1.2463x over previous
"""AttnBlock (GroupNorm + single-head 4096-token attention + proj + residual)
on 8 Trainium2 NeuronCores.

Sharding: core = (batch b = core//4, query-chunk qc = core%4).
Each core redundantly computes GroupNorm stats AND the full K/V for its
batch (K/V are needed by every query) directly from the x slab it already
loads for the stats — no collectives, no DRAM roundtrip for K/V.
Attention/proj run for the core's 1024 queries.

Precision plan (rel-err budget 2e-2):
  x slab arrives bf16 (halves the startup DMA), GroupNorm stats in fp32.
  Q/K/V projections, S=K^T.T@Q^T and O=V.T@E all run as fp8e4m3 DoubleRow
  matmuls (2 k-tiles per instruction, 2x PE rate): weights are quantized
  to fp8 on the host, h/K/Q/V/E quantize on the on-chip PSUM->SBUF copy.
  The output projection + residual stay fp32r/fp32.
  exp uses a constant -2 shift (softmax-invariant) so E fits fp8 range;
  numerator and denominator use the SAME quantized E (noise cancels).

All tensors are channel-major ([C, n]) on chip; layouts pack the
contraction pairs as [128, pair, free] so every DoubleRow operand is a
single strided AP. The softmax denominator accumulates on the DVE (idle
during attention) and is partition-summed by one fp32 ones matmul.
"""

import os
import sys

import ml_dtypes
import numpy as np

sys.path.insert(0, "/opt/trn_rl_repo")

import concourse.bass as bass
import concourse.bacc as bacc
import concourse.tile as tile
from concourse import mybir
from concourse.bass_utils import run_bass_kernel_spmd

F32 = mybir.dt.float32
F32R = mybir.dt.float32r
BF16 = mybir.dt.bfloat16
FP8 = mybir.dt.float8e4
DR = mybir.MatmulPerfMode.DoubleRow
AF = mybir.ActivationFunctionType
OP = mybir.AluOpType

B = 2
C = 512
N = 4096          # H*W tokens per batch
NQ = 1024         # queries per core
P = 128
NT = C // P       # 4 channel tiles
NCH = N // 512    # 8 column chunks of x
NJ = N // P       # 32 j-tiles
NPAIR = NJ // 2   # 16 j-tile pairs
EPS = 1e-6
SM_SCALE = float(C) ** -0.5
ESHIFT = -2.0     # exp shift: keeps E=exp(S/sqrt(C)-2) inside fp8e4m3
NCORES = 8

_CACHE = {}
USE_CC = False


def _emit(tc, t):
    """Emit the whole per-core kernel. `t` maps name -> DRAM tensor handle."""
    nc = tc.nc
    r = lambda ap: ap.bitcast(F32R)

    with (
        tc.tile_pool(name="consts", bufs=1) as consts,
        tc.tile_pool(name="xpool", bufs=1) as xpool,
        tc.tile_pool(name="ktpool", bufs=1) as ktpool,
        tc.tile_pool(name="vpool", bufs=1) as vpool,
        tc.tile_pool(name="qtpool", bufs=1) as qtpool,
        tc.tile_pool(name="ps", bufs=1, space="PSUM") as ps,
    ):
        # ---- constants -------------------------------------------------
        vecs = consts.tile([P, 20], F32)   # [nscale|nbias|bq|bk|bproj_eff] x4
        nc.sync.dma_start(out=vecs, in_=t["vecs"][:, :])
        memb = consts.tile([P, 8], F32)    # c -> group-in-tile one-hot
        nc.sync.dma_start(out=memb, in_=t["memb"][:, :])
        membT = consts.tile([8, P], F32)
        nc.sync.dma_start(out=membT, in_=t["membT"][:, :])
        ones_row = consts.tile([1, P], F32)
        nc.vector.memset(ones_row, 1.0)
        ones_col = consts.tile([P, 1], F32)
        nc.vector.memset(ones_col, 1.0)
        eshift = consts.tile([P, 1], F32)
        nc.vector.memset(eshift, ESHIFT)
        A_sb = consts.tile([P, NT], F32)   # per-channel scale (per tile col)
        B_sb = consts.tile([P, NT], F32)   # per-channel shift

        nsc = lambda tt: vecs[:, 0 * NT + tt:0 * NT + tt + 1]
        nbi = lambda tt: vecs[:, 1 * NT + tt:1 * NT + tt + 1]
        bq_ = lambda tt: vecs[:, 2 * NT + tt:2 * NT + tt + 1]
        bk_ = lambda tt: vecs[:, 3 * NT + tt:3 * NT + tt + 1]
        bpe = lambda tt: vecs[:, 4 * NT + tt:4 * NT + tt + 1]

        # ---- phase 1+2: stats, weights, Q^T, K^T, V --------------------
        xslab = [xpool.tile([P, N], BF16, tag=f"x{tt}", name=f"xs{tt}")
                 for tt in range(NT)]
        for q in range(4):
            for tt in range(NT):
                nc.sync.dma_start(
                    out=xslab[tt][:, q * 1024:(q + 1) * 1024],
                    in_=t["xT"][tt * P:(tt + 1) * P, q * 1024:(q + 1) * 1024])

        QT8 = qtpool.tile([P, NT, NQ], FP8, name="qt8")
        KT8 = ktpool.tile([P, NT, N], FP8, tag="kt8", name="kt8")
        V8 = [vpool.tile([P, 2, C], FP8, tag=f"v{i}", name=f"v{i}")
              for i in range(NPAIR)]

        with (
            tc.tile_pool(name="stream", bufs=1) as stream,
            tc.tile_pool(name="wkvpool", bufs=1) as wkvpool,
            tc.tile_pool(name="statsb", bufs=1) as statsb,
        ):
            def load_w8(dram, eng):
                w = wkvpool.tile([P, NT, C], FP8, tag=f"w{dram.name}",
                                 name=f"w{dram.name}")
                for cc in range(NT):
                    eng.dma_start(out=w[:, cc, :],
                                  in_=dram[cc * P:(cc + 1) * P, :])
                return w

            wq8 = load_w8(t["wq"], nc.gpsimd)
            wk8 = load_w8(t["wk"], nc.gpsimd)
            wv8 = load_w8(t["wv"], nc.gpsimd)

            # pass 1: stats split across DVE (ch 2-7) and ACT (ch 0-1)
            NDV = 6
            stats = [statsb.tile([P, NDV, 6], F32, tag=f"st{tt}", name=f"st{tt}")
                     for tt in range(NT)]
            s_extra = statsb.tile([P, NT, 2, 2], F32)   # [tt, unit, (s1, s2)]
            for ch in range(NCH):
                for tt in range(NT):
                    sl = xslab[tt][:, ch * 512:(ch + 1) * 512]
                    if ch >= 2:
                        nc.vector.bn_stats(out=stats[tt][:, ch - 2, :], in_=sl)
                    else:
                        u = ch
                        scr = stream.tile([P, 512], F32, tag="wraw1",
                                          name="ascr", bufs=1)
                        nc.scalar.activation(out=scr, in_=sl, func=AF.Copy,
                                             accum_out=s_extra[:, tt, u, 0:1])
                        scr2 = stream.tile([P, 512], F32, tag="wraw1",
                                           name="ascr2", bufs=1)
                        nc.scalar.activation(out=scr2, in_=sl, func=AF.Square,
                                             accum_out=s_extra[:, tt, u, 1:2])
            mvals = statsb.tile([P, NT, 2], F32)  # mean | E[x^2] per channel
            nsamp_d = float(NDV * 512)
            for tt in range(NT):
                mv = statsb.tile([P, 2], F32, tag="mv", name="mv")
                nc.vector.bn_aggr(out=mv, in_=stats[tt])
                # s1/s2 from the DVE span
                sd = statsb.tile([P, 2], F32, tag="sd", name="sd")
                nc.vector.tensor_scalar_mul(sd[:, 0:1], mv[:, 0:1], nsamp_d)
                msq = statsb.tile([P, 1], F32, tag="msq", name="msq")
                nc.vector.tensor_mul(msq, mv[:, 0:1], mv[:, 0:1])
                nc.vector.tensor_add(sd[:, 1:2], mv[:, 1:2], msq)
                nc.vector.tensor_scalar_mul(sd[:, 1:2], sd[:, 1:2], nsamp_d)
                # add the ACT partial sums
                tot = statsb.tile([P, 2], F32, tag="tot", name="tot")
                nc.vector.tensor_add(tot, sd, s_extra[:, tt, 0, :])
                nc.vector.tensor_add(tot, tot, s_extra[:, tt, 1, :])
                nc.vector.tensor_scalar_mul(mvals[:, tt, :], tot, 1.0 / 4096.0)
            # group reduction via tiny fp32 matmuls with the membership matrix
            psG = ps.tile([8, 2 * NT], F32, tag="st", name="psG", bufs=2)
            for tt in range(NT):
                nc.tensor.matmul(psG[:, tt:tt + 1], memb, mvals[:, tt, 0:1],
                                 start=True, stop=True)
                nc.tensor.matmul(psG[:, NT + tt:NT + tt + 1], memb,
                                 mvals[:, tt, 1:2], start=True, stop=True)
            MU = statsb.tile([8, NT], F32)
            QQ = statsb.tile([8, NT], F32)
            nc.vector.tensor_scalar_mul(MU, psG[:, 0:NT], 1.0 / 16.0)
            nc.vector.tensor_scalar_mul(QQ, psG[:, NT:2 * NT], 1.0 / 16.0)
            VAR = statsb.tile([8, NT], F32)
            nc.vector.tensor_mul(VAR, MU, MU)
            nc.vector.tensor_sub(VAR, QQ, VAR)
            SD = statsb.tile([8, NT], F32)
            eps_t = statsb.tile([8, 1], F32)
            nc.vector.memset(eps_t, EPS)
            nc.scalar.activation(out=SD, in_=VAR, func=AF.Sqrt, bias=eps_t)
            RSTD = statsb.tile([8, NT], F32)
            nc.vector.reciprocal(RSTD, SD)
            for tt in range(NT):
                psbc = ps.tile([P, 2], F32, tag="st", name="psbc", bufs=2)
                nc.tensor.matmul(psbc[:, 0:1], membT, RSTD[:, tt:tt + 1],
                                 start=True, stop=True)
                nc.tensor.matmul(psbc[:, 1:2], membT, MU[:, tt:tt + 1],
                                 start=True, stop=True)
                nc.vector.tensor_mul(A_sb[:, tt:tt + 1], psbc[:, 0:1], nsc(tt))
                tmp = statsb.tile([P, 1], F32, tag="tmp", name="tmp")
                nc.vector.tensor_mul(tmp, psbc[:, 1:2], A_sb[:, tt:tt + 1])
                nc.vector.tensor_sub(B_sb[:, tt:tt + 1], nbi(tt), tmp)

            # pass 2: per 512-token chunk: normalize to fp8 h, project K/V
            # (+Q for the local chunks 0-1) as fp8 DoubleRow pairs.
            for ch in range(NCH):
                h8 = stream.tile([P, NT, 512], FP8, tag=f"h{ch % 2}",
                                 name="h8", bufs=1)
                for tt in range(NT):
                    if tt < 2:
                        nc.vector.tensor_scalar(
                            out=h8[:, tt, :],
                            in0=xslab[tt][:, ch * 512:(ch + 1) * 512],
                            scalar1=A_sb[:, tt:tt + 1],
                            scalar2=B_sb[:, tt:tt + 1],
                            op0=OP.mult, op1=OP.add)
                    else:
                        nc.scalar.activation(
                            out=h8[:, tt, :],
                            in_=xslab[tt][:, ch * 512:(ch + 1) * 512],
                            func=AF.Identity,
                            bias=B_sb[:, tt:tt + 1],
                            scale=A_sb[:, tt:tt + 1])
                # K^T for this chunk
                for o in range(NT):
                    pk = ps.tile([P, 512], F32, tag="st", name="pk", bufs=2)
                    for op in range(2):
                        nc.tensor.matmul(
                            pk, wk8[:, 2 * op:2 * op + 2, o * P:(o + 1) * P],
                            h8[:, 2 * op:2 * op + 2, :],
                            start=(op == 0), stop=(op == 1), perf_mode=DR)
                    dst = KT8[:, o, ch * 512:(ch + 1) * 512]
                    if o < 2:
                        nc.vector.tensor_scalar_add(dst, pk, bk_(o))
                    else:
                        nc.scalar.activation(out=dst, in_=pk,
                                             func=AF.Identity, bias=bk_(o))
                # V for this chunk
                for nb in range(4):
                    pv = ps.tile([P, C], F32, tag="st", name="pv", bufs=2)
                    for op in range(2):
                        nc.tensor.matmul(
                            pv,
                            h8[:, 2 * op:2 * op + 2, nb * P:(nb + 1) * P],
                            wv8[:, 2 * op:2 * op + 2, :],
                            start=(op == 0), stop=(op == 1), perf_mode=DR)
                    dst = V8[ch * 2 + nb // 2][:, nb % 2, :]
                    if nb < 2:
                        nc.vector.tensor_copy(out=dst, in_=pv)
                    else:
                        nc.scalar.copy(out=dst, in_=pv)
                # Q^T for the local chunks
                if ch < 2:
                    for o in range(NT):
                        pq = ps.tile([P, 512], F32, tag="st", name="pq", bufs=2)
                        for op in range(2):
                            nc.tensor.matmul(
                                pq, wq8[:, 2 * op:2 * op + 2, o * P:(o + 1) * P],
                                h8[:, 2 * op:2 * op + 2, :],
                                start=(op == 0), stop=(op == 1), perf_mode=DR)
                        dst = QT8[:, o, ch * 512:(ch + 1) * 512]
                        if o < 2:
                            nc.vector.tensor_scalar_add(dst, pq, bq_(o))
                        else:
                            nc.scalar.activation(out=dst, in_=pq,
                                                 func=AF.Identity, bias=bq_(o))

        # ---- phase 3: attention + output projection --------------------
        with (
            tc.tile_pool(name="attnsb", bufs=2) as attnsb,
            tc.tile_pool(name="epool", bufs=2) as epool,
        ):
            wproj_sb = []
            for cc in range(NT):
                w = attnsb.tile([P, C], F32R, tag=f"wp{cc}", name=f"wp{cc}", bufs=1)
                nc.sync.dma_start(out=w, in_=t["wproj"][cc * P:(cc + 1) * P, :])
                wproj_sb.append(w)
            for ih in range(NQ // 512):
                i0 = ih * 512
                # residual tiles for this half, straight from the x slab
                res_t = []
                for o in range(NT):
                    res = attnsb.tile([P, 512], F32, tag=f"res{o}", name=f"res{o}", bufs=1)
                    nc.vector.tensor_scalar_add(
                        res, xslab[o][:, i0:i0 + 512], bpe(o))
                    res_t.append(res)
                ps_ot = [ps.tile([P, 512], F32, tag=f"ot{c}", name=f"ot{c}")
                         for c in range(NT)]
                acc = attnsb.tile([P, 512], F32, tag="acc", name="acc")

                def do_S(pr):
                    """S^T for both j-tiles of pair pr + one batched exp."""
                    e8 = epool.tile([P, 2, 512], FP8, tag="e", name="e")
                    ps_st = ps.tile([P, 2, 512], F32, tag="st", name="st",
                                    bufs=2)
                    for half in range(2):
                        jt = pr * 2 + half
                        for op in range(2):
                            nc.tensor.matmul(
                                ps_st[:, half, :],
                                KT8[:, 2 * op:2 * op + 2, jt * P:(jt + 1) * P],
                                QT8[:, 2 * op:2 * op + 2, i0:i0 + 512],
                                start=(op == 0), stop=(op == 1), perf_mode=DR)
                    nc.scalar.activation(out=e8, in_=ps_st, func=AF.Exp,
                                         scale=SM_SCALE, bias=eshift)
                    return e8

                pend = do_S(0)
                for pr in range(NPAIR):
                    # prefetch next pair's S while this pair's O runs
                    nxt = do_S(pr + 1) if pr + 1 < NPAIR else None
                    e8 = pend
                    first, last = (pr == 0), (pr == NPAIR - 1)
                    for c in range(NT):
                        nc.tensor.matmul(ps_ot[c],
                                         V8[pr][:, :, c * P:(c + 1) * P],
                                         e8, start=first, stop=last,
                                         perf_mode=DR)
                    # denominator partials accumulate on the (idle) DVE
                    if first:
                        nc.vector.tensor_add(acc, e8[:, 0, :], e8[:, 1, :])
                    else:
                        nc.vector.tensor_add(acc, acc, e8[:, 0, :])
                        nc.vector.tensor_add(acc, acc, e8[:, 1, :])
                    pend = nxt
                # softmax denominator: partition-sum, reciprocal, bcast
                ps_d = ps.tile([1, 512], F32, tag="st", name="psd", bufs=2)
                nc.tensor.matmul(ps_d, ones_col, acc, start=True, stop=True)
                d_sb = attnsb.tile([1, 512], F32, tag="dsb", name="dsb")
                nc.vector.tensor_copy(out=d_sb, in_=ps_d)
                dr_sb = attnsb.tile([1, 512], F32, tag="drsb", name="drsb")
                nc.vector.reciprocal(dr_sb, d_sb)
                ps_b = ps.tile([P, 512], F32, tag="st", name="psb", bufs=2)
                nc.tensor.matmul(ps_b, ones_row, dr_sb, start=True, stop=True)
                db_sb = attnsb.tile([P, 512], F32, tag="db", name="db", bufs=1)
                nc.vector.tensor_copy(out=db_sb, in_=ps_b)
                # normalize O^T
                ot_sb = []
                for c in range(NT):
                    o_sb = attnsb.tile([P, 512], F32, tag=f"osb{c}", name=f"osb{c}", bufs=1)
                    nc.vector.tensor_mul(r(o_sb), ps_ot[c], db_sb)
                    ot_sb.append(o_sb)
                # output projection + bias + residual
                for o in range(NT):
                    ps_o = ps.tile([P, 512], F32, tag="st", name="ps_o", bufs=2)
                    for cc in range(NT):
                        nc.tensor.matmul(ps_o,
                                         r(wproj_sb[cc][:, o * P:(o + 1) * P]),
                                         r(ot_sb[cc]),
                                         start=(cc == 0), stop=(cc == NT - 1))
                    outt = attnsb.tile([P, 512], F32, tag="outt", name="outt")
                    nc.vector.tensor_add(outt, ps_o, res_t[o])
                    nc.sync.dma_start(
                        out=t["outT"][o * P:(o + 1) * P, i0:i0 + 512], in_=outt)


def _build_nc():
    nc = bacc.Bacc("TRN2", target_bir_lowering=False, debug=False)
    dp = nc.declare_dram_parameter
    t = {
        "xT": dp("xT", [C, N], BF16, isOutput=False),
        "wq": dp("wq", [C, C], FP8, isOutput=False),
        "wk": dp("wk", [C, C], FP8, isOutput=False),
        "wv": dp("wv", [C, C], FP8, isOutput=False),
        "wproj": dp("wproj", [C, C], F32R, isOutput=False),
        "vecs": dp("vecs", [P, 20], F32, isOutput=False),
        "memb": dp("memb", [P, 8], F32, isOutput=False),
        "membT": dp("membT", [8, P], F32, isOutput=False),
        "outT": dp("outT", [C, NQ], F32, isOutput=True),
    }
    with tile.TileContext(nc, num_cores=NCORES) as tc:
        _emit(tc, t)
    nc.finalize()
    return nc


def get_nc():
    if "nc" not in _CACHE:
        _CACHE["nc"] = _build_nc()
    return _CACHE["nc"]


def prep_in_maps(x, norm_scale, norm_bias, wq, bq, wk, bk, wv, bv, wproj, bproj):
    f = lambda a: np.ascontiguousarray(np.asarray(a), dtype=np.float32)
    x = f(x)
    wproj = f(wproj)
    q8 = lambda a: np.ascontiguousarray(f(a).astype(ml_dtypes.float8_e4m3))
    wq8, wk8, wv8 = q8(wq), q8(wk), q8(wv)
    bproj_eff = f(bproj) + f(bv) @ wproj
    vecs = np.zeros((P, 20), np.float32)
    for idx, v in enumerate([f(norm_scale), f(norm_bias), f(bq), f(bk), bproj_eff]):
        vecs[:, idx * NT:(idx + 1) * NT] = v.reshape(NT, P).T
    memb = np.zeros((P, 8), np.float32)
    memb[np.arange(P), np.arange(P) // 16] = 1.0
    membT = np.ascontiguousarray(memb.T)
    xr = x.reshape(B, N, C)
    in_maps = []
    xT_cache = {}
    for core in range(NCORES):
        b, qc = divmod(core, 4)
        if b not in xT_cache:
            xT_cache[b] = np.ascontiguousarray(xr[b].T)
        s = qc * NQ
        xTb = xT_cache[b]
        xT_rot = np.ascontiguousarray(
            np.concatenate([xTb[:, s:], xTb[:, :s]], axis=1)
            .astype(ml_dtypes.bfloat16))
        in_maps.append({
            "xT": xT_rot, "wq": wq8, "wk": wk8, "wv": wv8,
            "wproj": wproj, "vecs": vecs, "memb": memb, "membT": membT,
        })
    return in_maps


def assemble(results):
    out = np.empty((B, N, C), np.float32)
    for core in range(NCORES):
        b, qc = divmod(core, 4)
        out[b, qc * NQ:(qc + 1) * NQ, :] = results[core]["outT"].T
    return out.reshape(B, 64, 64, C)


def run(trace=False, **inputs):
    nc = get_nc()
    in_maps = prep_in_maps(**inputs)
    res = run_bass_kernel_spmd(nc, in_maps, list(range(NCORES)), trace=trace)
    return assemble(res.results), res


def kernel(**inputs):
    out, _ = run(trace=False, **inputs)
    return out
